# revision 2
# baseline (speedup 1.0000x reference)
"""Single-head attention kernel v2 for Trainium2 (8 NeuronCores, SPMD).

Problem: x[4,4096,1024] f32, padding_mask[4,1,4096] i32, Wk/Wq/Wv[64,1024] f32.
  k/q/v = x @ W.T ; wei = softmax(mask(q k^T / 8)) ; out = wei @ v  -> [4,4096,64]

Sharding: core c = (b = c//2, half = c%2) owns tokens [half*2048,(half+1)*2048)
of batch b: it reads ONLY those rows of x (4MB as bf16), computes k/q/v for
them, then exchanges k/v with its sibling via a pairwise AllGather so both
halves see all 4096 keys (global key order = group-rank order).  Queries stay
local -> each core returns out rows [2048, 64].

Per-core pipeline (matmuls bf16/fp8 -- f32r is power-throttled ~4x on trn2):
  1. x arrives pre-converted to bf16; DMA-XBAR transpose loads xT [c,t] into
     SBUF directly.  Zero PE transposes in the whole kernel.
  2. Projections: stationary [wq|wk] [128c,128] per cc chunk, moving xT
     [128c,512t] -> PSUM rows 0:64 = q, 64:128 = k, accumulated over 8 cc.
     v: stationary wv [128c,64] -> vT [64,512], two token-blocks per bank.
  3. Copies: q -> qT3 fp8 [64,qb,2,512] (DoubleRow j=1 half zeroed),
     k -> kT3 fp8 rows 64:128, vT -> bf16 staging -> DMA-XBAR transpose ->
     v_sb [keys,65] (col 64 = ones for softmax denominators).
  4. Exchange: own kT/v slab -> DRAM (byte-packed), pairwise AllGather,
     both rank slabs DMAed back in rank order (= global key order).
  5. Phase 2, kc-major over 32 key chunks, 4 query blocks inner:
     scores sT[128k,512q] via fp8 DoubleRow (2x row rate), exp on ACT
     (scale 1/8, per-key bias -1e5*(1-mask): masked keys underflow to
     exactly 0) -> bf16, PV oT[65,512] += v^T exp over all kc.
  6. Epilogue per qb: oT -> bf16 [80,512] -> DMA-XBAR transpose -> [q,65],
     multiply by 1/denominator and query mask -> f32 out.
"""

import sys

if "/opt/trn_rl_repo" not in sys.path:
    sys.path.insert(0, "/opt/trn_rl_repo")

import numpy as np
import ml_dtypes

import concourse.bass as bass
import concourse.mybir as mybir
import concourse.tile as tile
from concourse import bacc
from concourse.bass_utils import run_bass_kernel_spmd

F32 = mybir.dt.float32
BF16 = mybir.dt.bfloat16
FP8 = mybir.dt.float8e4
DR = mybir.MatmulPerfMode.DoubleRow

T = 4096
TL = 2048
C = 1024
H = 64
NCC = 8
NKC = 32
NKCL = 16
NQB = 4
NTT = TL // 128   # 16
NEG = -1.0e5

KBYTES = NKCL * 128 * 2          # 4096 bytes/partition of own kT slab (bf16)
VBYTES = NKCL * 66 * 2           # 2112 bytes/partition of own v slab

USE_DR = False
USE_NOLOAD = False
DEBUG = False


def mm_noload(nc, out, lhsT, rhs, start=True, stop=True, perf_mode=None,
              tile_position=None):
    te = nc.tensor
    keep = {0, 1} if perf_mode == DR else {0}
    ifmap_ap = te.lower_ap(rhs.opt(keep), opt=False)
    weights_ap = te.lower_ap(lhsT.opt(keep), opt=False, for_matmul_weights=True)
    out_ap = te.lower_ap(out)
    if tile_position is None:
        tile_position = (rhs.base_partition(), out.base_partition())
    return te.add_instruction(
        mybir.InstMatmult(
            name=f"I-{nc.next_id()}",
            replication_resolution=0,
            replication_shift_amnt=0,
            replication_num_rows=0,
            start_tensor_calc=start,
            stop_tensor_calc=stop,
            ins=[ifmap_ap, weights_ap],
            outs=[out_ap],
            perf_mode=perf_mode,
            is_transpose=None,
            tile_position=tile_position,
            tile_size=(128, 128),
            ldweights=False,
        )
    )


def build_nc():
    nc = bacc.Bacc("TRN2", target_bir_lowering=False, debug=False, num_devices=8)

    x_d = nc.dram_tensor("x", [TL, C], BF16, kind="ExternalInput")
    wkq_d = nc.dram_tensor("wkq", [128, NCC, 128], BF16, kind="ExternalInput")
    wv_d = nc.dram_tensor("wv", [128, NCC, H], BF16, kind="ExternalInput")
    nbias_d = nc.dram_tensor("nbias", [128, NKC], F32, kind="ExternalInput")
    maskq_d = nc.dram_tensor("maskq", [128, NTT], F32, kind="ExternalInput")
    out_d = nc.dram_tensor("out", [TL, H], F32, kind="ExternalOutput")
    dbg = {}
    if DEBUG:
        dbg["kT"] = nc.dram_tensor("dbg_kT", [64, NKC * 128], BF16, kind="ExternalOutput")
        dbg["qT"] = nc.dram_tensor("dbg_qT", [64, NQB * 512], BF16, kind="ExternalOutput")
        dbg["v"] = nc.dram_tensor("dbg_v", [128, NKC * 66], BF16, kind="ExternalOutput")
        dbg["oTT"] = nc.dram_tensor("dbg_oTT", [128, NTT * 80], BF16, kind="ExternalOutput")
        dbg["sT"] = nc.dram_tensor("dbg_sT", [128, 512], F32, kind="ExternalOutput")
        dbg["exp"] = nc.dram_tensor("dbg_exp", [128, 512], BF16, kind="ExternalOutput")
        dbg["oT"] = nc.dram_tensor("dbg_oT", [80, 512], BF16, kind="ExternalOutput")

    with tile.TileContext(nc) as tc:
        with (
            tc.tile_pool(name="const", bufs=1) as const,
            tc.tile_pool(name="persist", bufs=1) as persist,
            tc.tile_pool(name="expp", bufs=6) as expp,
            tc.tile_pool(name="osb", bufs=2) as osb,
            tc.tile_pool(name="small", bufs=4) as small,
            tc.tile_pool(name="P1", bufs=4, space=bass.MemorySpace.PSUM) as P1,
            tc.tile_pool(name="P2", bufs=4, space=bass.MemorySpace.PSUM) as P2,
            tc.tile_pool(name="dram", bufs=1, space="DRAM") as dram,
        ):
            _emit(nc, const, persist, expp, osb, small, P1, P2, dram,
                  x_d, wkq_d, wv_d, nbias_d, maskq_d, out_d, dbg)

    nc.compile()
    return nc


def _emit(nc, const, persist, expp, osb, small, P1, P2, dram,
          x_d, wkq_d, wv_d, nbias_d, maskq_d, out_d, dbg=None):
    # ---------------- constants / persistent tiles ----------------
    wkq_sb = const.tile([128, NCC, 128], BF16)
    wv_sb = const.tile([128, NCC, H], BF16)
    nbias_sb = const.tile([128, NKC], F32)
    maskq_sb = const.tile([128, NTT], F32)
    nc.gpsimd.dma_start(out=wkq_sb, in_=wkq_d.ap())
    nc.gpsimd.dma_start(out=wv_sb, in_=wv_d.ap())
    nc.gpsimd.dma_start(out=nbias_sb, in_=nbias_d.ap())
    nc.gpsimd.dma_start(out=maskq_sb, in_=maskq_d.ap())

    xT_sb = persist.tile([128, NCC, TL], BF16)
    kT3 = persist.tile([64, NKC, 128], BF16)
    kstage = persist.tile([128, 4 * 128], BF16)     # k hop: PSUM rows 64:128
    qT3 = persist.tile([64, NQB, 512], BF16)
    v_sb = persist.tile([128, NKC, 66], BF16)       # [key, 65(+pad)]
    vT_sb = persist.tile([128, 2 * 512], BF16)      # vT staging, parity rows
    oTT = persist.tile([128, NTT, 80], BF16)
    out_acc = persist.tile([128, NTT, H], F32)

    ones_sb = const.tile([128, NKC], BF16)
    nc.gpsimd.memset(ones_sb, 1.0)
    nc.gpsimd.tensor_copy(v_sb[:, :, 64], ones_sb)

    # ---------------- 1) x^T via DMA-XBAR transpose ----------------
    # NOTE: all XBAR transposes go through one engine queue (nc.sync):
    # concurrent XBAR use from two queues corrupts output (shared unit).
    for cc in range(NCC):
        nc.sync.dma_start_transpose(
            xT_sb[:, cc, :], x_d.ap()[:, cc * 128:(cc + 1) * 128]
        )

    # ---------------- 2) projections ----------------
    kq_ps = [P1.tile([128, 512], F32, tag="s", name=f"kq{i}") for i in range(4)]
    vt_ps = [P2.tile([128, 512], F32, tag="o", name=f"vt{i}") for i in range(2)]
    for cc in range(NCC):
        first, last = cc == 0, cc == NCC - 1
        for tb in range(4):
            nc.tensor.matmul(
                kq_ps[tb],
                wkq_sb[:, cc, :],
                xT_sb[:, cc, tb * 512:(tb + 1) * 512],
                start=first, stop=last,
            )
        for tb in range(4):
            r0 = (tb % 2) * 64
            nc.tensor.matmul(
                vt_ps[tb // 2][r0:r0 + 64, :],
                wv_sb[:, cc, :],
                xT_sb[:, cc, tb * 512:(tb + 1) * 512],
                start=first, stop=last,
            )

    # ---------------- 3) copies into phase-2 layouts (local slots) --------
    def _copy(eng, out, in_):
        if eng is nc.scalar:
            eng.copy(out, in_)
        else:
            eng.tensor_copy(out, in_)

    engines = [nc.vector, nc.scalar]
    for tb in range(4):
        _copy(engines[tb % 2], qT3[:, tb, :], kq_ps[tb][0:64, :])
        # k: PSUM rows 64:128 -> fp8 staging (same partitions), then a
        # SBUF->SBUF DMA hop down to partitions 0:64 (engines can't cross
        # partitions; DMA can)
        _copy(engines[(tb + 1) % 2],
            kstage[64:128, :],
            kq_ps[tb][64:128, :],
        )
        nc.gpsimd.dma_start(
            out=kT3[:, 4 * tb:4 * tb + 4, :],
            in_=kstage[64:128, :].rearrange("p (kc f) -> p kc f", kc=4),
        )
        r0 = (tb % 2) * 64
        _copy(engines[tb % 2],
            vT_sb[r0:r0 + 64, (tb // 2) * 512:(tb // 2 + 1) * 512],
            vt_ps[tb // 2][r0:r0 + 64, :],
        )
    # one XBAR transpose per 128-key chunk keeps the natural key order
    # (a multi-chunk out AP would interleave tokens partition-major).
    # XBAR needs a CONTIGUOUS destination -> land in v_stg, then copy into
    # the strided v_sb[:, kc, 0:64] slots on Pool/DVE.
    v_stg = persist.tile([128, NKCL, 64], BF16)
    for tb in range(4):
        r0 = (tb % 2) * 64
        c0 = (tb // 2) * 512
        for kcl in range(4):
            nc.sync.dma_start_transpose(
                v_stg[:, 4 * tb + kcl, :],
                vT_sb[r0:r0 + 64, c0 + kcl * 128:c0 + (kcl + 1) * 128],
            )
    nc.gpsimd.tensor_copy(v_sb[:, 0:NKCL, 0:64], v_stg)

    # ---------------- 4) sibling exchange ----------------
    # byte-packed bounce: rows 0:64 cols [0:KBYTES) = kT slab,
    # all 128 rows cols [KBYTES:KBYTES+VBYTES) = v slab.
    bounce_in = dram.tile([128, KBYTES + VBYTES], mybir.dt.uint8)
    bounce_out = dram.tile([256, KBYTES + VBYTES], mybir.dt.uint8)
    nc.gpsimd.dma_start(
        out=bounce_in[0:64, 0:KBYTES],
        in_=kT3[:, 0:NKCL, :].rearrange("p a f -> p (a f)")
        .bitcast(mybir.dt.uint8),
    )
    nc.gpsimd.dma_start(
        out=bounce_in[:, KBYTES:KBYTES + VBYTES],
        in_=v_sb[:, 0:NKCL, :].rearrange("p a f -> p (a f)")
        .bitcast(mybir.dt.uint8),
    )
    groups = [[0, 1], [2, 3], [4, 5], [6, 7]]
    nc.gpsimd.collective_compute(
        "AllGather", mybir.AluOpType.bypass, replica_groups=groups,
        ins=[bounce_in.opt()], outs=[bounce_out.opt()],
    )
    # write back BOTH rank slabs in rank order -> global key order
    for r in range(2):
        nc.gpsimd.dma_start(
            out=kT3[:, r * NKCL:(r + 1) * NKCL, :]
            .rearrange("p a f -> p (a f)").bitcast(mybir.dt.uint8),
            in_=bounce_out[r * 128:r * 128 + 64, 0:KBYTES],
        )
        nc.gpsimd.dma_start(
            out=v_sb[:, r * NKCL:(r + 1) * NKCL, :]
            .rearrange("p a f -> p (a f)").bitcast(mybir.dt.uint8),
            in_=bounce_out[r * 128:(r + 1) * 128, KBYTES:KBYTES + VBYTES],
        )

    if DEBUG:
        nc.gpsimd.dma_start(out=dbg["kT"].ap(), in_=kT3.rearrange("p a f -> p (a f)"))
        nc.gpsimd.dma_start(out=dbg["qT"].ap(), in_=qT3.rearrange("p a f -> p (a f)"))
        nc.gpsimd.dma_start(out=dbg["v"].ap(), in_=v_sb.rearrange("p a f -> p (a f)"))

    # ---------------- 5) phase 2 ----------------
    oT_ps = [P2.tile([128, 512], F32, tag="o", name=f"oT{i}") for i in range(NQB)]
    for kc in range(NKC):
        sT = []
        for qb in range(NQB):
            s = P1.tile([128, 512], F32, tag="s", name="sT")
            nc.tensor.matmul(
                s, kT3[:, kc, :], qT3[:, qb, :],
                start=True, stop=True,
            )
            sT.append(s)
        if DEBUG and kc == 0:
            dbg_s = small.tile([128, 512], F32, name="dbgs")
            nc.vector.tensor_copy(dbg_s, sT[0])
            nc.gpsimd.dma_start(out=dbg["sT"].ap(), in_=dbg_s)
        exps = []
        for qb in range(NQB):
            e = expp.tile([128, 512], BF16, name="exp")
            nc.scalar.activation(
                e, sT[qb], mybir.ActivationFunctionType.Exp,
                bias=nbias_sb[:, kc:kc + 1], scale=0.125,
            )
            exps.append(e)
        if DEBUG and kc == 0:
            nc.gpsimd.dma_start(out=dbg["exp"].ap(), in_=exps[0])
        for qb in range(NQB):
            nc.tensor.matmul(
                oT_ps[qb][0:65, :], v_sb[:, kc, 0:65], exps[qb],
                start=(kc == 0), stop=(kc == NKC - 1),
            )

    # ---------------- 6) epilogue ----------------
    # XBAR fold for 512 columns is block-major: oTT[p, 4*qb+a, j] =
    # oTs[j, a*128+p], i.e. query qb*512 + a*128 + p -- the natural layout.
    for qb in range(NQB):
        oTs = osb.tile([80, 512], BF16)
        nc.vector.tensor_copy(oTs[0:65, :], oT_ps[qb][0:65, :])
        if DEBUG and qb == 0:
            nc.gpsimd.dma_start(out=dbg["oT"].ap(), in_=oTs)
        nc.sync.dma_start_transpose(oTT[:, 4 * qb:4 * qb + 4, :], oTs)
    for tt in range(NTT):
        recip = small.tile([128, 1], F32)
        nc.vector.reciprocal(recip, oTT[:, tt, 64:65])
        nc.vector.tensor_scalar(
            out=out_acc[:, tt, :],
            in0=oTT[:, tt, 0:64],
            scalar1=recip,
            scalar2=maskq_sb[:, tt:tt + 1],
            op0=mybir.AluOpType.mult,
            op1=mybir.AluOpType.mult,
        )
    if DEBUG:
        nc.gpsimd.dma_start(out=dbg["oTT"].ap(), in_=oTT.rearrange("p a f -> p (a f)"))
    nc.gpsimd.dma_start(
        out=out_d.ap().rearrange("(n p) h -> p n h", p=128), in_=out_acc
    )


_NC_CACHE = None


def _get_nc():
    global _NC_CACHE
    if _NC_CACHE is None:
        _NC_CACHE = build_nc()
    return _NC_CACHE


def make_in_maps(x, padding_mask, Wk, Wq, Wv):
    x = np.asarray(x)
    padding_mask = np.asarray(padding_mask)

    def wt(w):  # [64,1024] -> [128, 8, 64]: wt[p, cc, h] = w[h, cc*128+p]
        return np.ascontiguousarray(
            np.asarray(w).T.reshape(NCC, 128, H).transpose(1, 0, 2)
        )

    wkt, wqt, wvt = wt(Wk), wt(Wq), wt(Wv)
    # stationary [wq | wk] -> psum rows 0:64 = q, 64:128 = k
    wkq = np.concatenate([wqt, wkt], axis=2).astype(ml_dtypes.bfloat16)
    wv = wvt.astype(ml_dtypes.bfloat16)

    in_maps = []
    for core in range(8):
        b, half = core // 2, core % 2
        xb = np.ascontiguousarray(
            x[b, half * TL:(half + 1) * TL]
        ).astype(ml_dtypes.bfloat16)
        m = padding_mask[b, 0].astype(np.float32)          # global key mask
        nbias = np.ascontiguousarray(
            (NEG * (1.0 - m)).reshape(NKC, 128).T
        )
        mloc = m[half * TL:(half + 1) * TL]
        maskq = np.ascontiguousarray(mloc.reshape(NTT, 128).T)
        in_maps.append({
            "x": xb, "wkq": wkq, "wv": wv,
            "nbias": nbias, "maskq": maskq,
        })
    return in_maps


def kernel(x, padding_mask, Wk, Wq, Wv):
    nc = _get_nc()
    in_maps = make_in_maps(x, padding_mask, Wk, Wq, Wv)
    res = run_bass_kernel_spmd(nc, in_maps, core_ids=list(range(8)), trace=False)
    B = np.asarray(x).shape[0]
    out = np.empty((B, T, H), dtype=np.float32)
    for c in range(8):
        b, half = c // 2, c % 2
        out[b, half * TL:(half + 1) * TL, :] = res.results[c]["out"]
    return out
